# revision 1
# baseline (speedup 1.0000x reference)
"""AuthPct metric kernel for 8 Trainium2 NeuronCores.

Sharding: real_stats rows are sharded across the 8 cores (1536 each).
For column features f_j each core computes PSUM tiles of

    Y[j, i] = 2*f_j.r_i - |r_i|^2 - |f_j|^2  =  -dist^2(f_j, r_i)

via bf16 PE matmuls: two K=128 feature chunks plus one augmented K=128
matmul (rows 0..3 of its operands carry the exact hi/lo bf16 splits of
-|r_i|^2 and -|f_j|^2; remaining rows are zero).  For the gen side the
aug lhsT has only the |r_i|^2 rows, so gen tiles hold X = 2G - |r_i|^2.

gen (96 j-tiles/core, all gen columns vs core rows): ScalarE copies
PSUM->SBUF wide [128,1536]; DVE `max` top-8 (d1 values) + `max_index`
(argmin payload for d2).  The 60 real j-tiles are spread evenly over
the 96 loop slots so the DVE (bottleneck) sees uniform demand.

real: the distance matrix is symmetric, so each unordered shard pair is
computed once.  With host-rotated real columns every core runs the SAME
program on j-tiles covering shards c..c+4 (60 j-tiles): DVE `max` top-8
gives the j-side min (the diagonal lands in the self block where
Y_diag ~ 0 while true neighbors are ~ -300, so host uses top-2 there);
Pool `partition_all_reduce(max)` on blocks c+1..c+4 gives the i-side
min over each tile's 128 j's, and the j-side DVE scans run only on
blocks c..c+3 (free-side coverage s-3..s plus PAR coverage s+1..s+4
spans all 8 shards).  This cuts real-side DVE scans from 96 to 48
j-tiles; the partition reduces ride on the otherwise-idle Pool engine.
Measured ~429 us HW exec; the DVE scans run back-to-back (~98% busy,
<1 us total idle), so the span is the scan floor plus ramp and the
fixed Tile tail barrier.

Host combines the per-core partials (min over all candidates), gathers
d2 = realNN[argmin], applies sigmoid and the mean.  All reductions are
exact fp32; only the Gram matmuls are bf16.
"""

import numpy as np

N = 12288
D = 256
NCORES = 8
SHARD = N // NCORES          # 1536 rows per core
JTILE = 128                  # j columns per tile (PSUM partitions)
NJT = N // JTILE             # 96 gen j-tiles
RJT = 60                     # real j-tiles: shards c..c+4 (rotated)
FJT = 48                     # j-tiles with a DVE free-side scan (m=0..3)
PAR_LO, PAR_HI = 12, 60      # real j-tiles with partition-reduce harvest
NT = 512                     # i elements per matmul (PSUM bank)
NIT = SHARD // NT            # 3 i-tiles

_cached_nc = None


def _build_nc():
    import concourse.bass_isa as bass_isa
    import concourse.mybir as mybir
    from concourse import bacc
    from concourse.tile import TileContext

    f32 = mybir.dt.float32
    bf16 = mybir.dt.bfloat16
    u32 = mybir.dt.uint32

    nc = bacc.Bacc("TRN2", target_bir_lowering=False, debug=False,
                   num_devices=NCORES)

    colr = nc.dram_tensor("colr", [D, RJT * JTILE], bf16,
                          kind="ExternalInput")
    colg = nc.dram_tensor("colg", [D, N], bf16, kind="ExternalInput")
    auglr = nc.dram_tensor("auglr", [JTILE, RJT * JTILE], bf16,
                           kind="ExternalInput")
    rhs = nc.dram_tensor("rhs", [D, SHARD], bf16, kind="ExternalInput")
    aug = nc.dram_tensor("aug", [128, SHARD], bf16, kind="ExternalInput")
    ones = nc.dram_tensor("ones", [JTILE, JTILE], bf16, kind="ExternalInput")

    o_realv = nc.dram_tensor("o_realv", [128, FJT * 8], f32,
                             kind="ExternalOutput")
    o_par = nc.dram_tensor("o_par", [PAR_HI - PAR_LO, SHARD], f32,
                           kind="ExternalOutput")
    o_genv = nc.dram_tensor("o_genv", [128, NJT * 8], f32,
                            kind="ExternalOutput")
    o_geni = nc.dram_tensor("o_geni", [128, NJT * 8], u32,
                            kind="ExternalOutput")

    with TileContext(nc) as tc:
        with (
            tc.tile_pool(name="const", bufs=1) as constp,
            tc.tile_pool(name="lhs", bufs=6) as lhsp,
            tc.tile_pool(name="wide", bufs=6) as widep,
            tc.tile_pool(name="parp", bufs=3) as parp,
            tc.tile_pool(name="outb", bufs=1) as outp,
            tc.tile_pool(name="ps", bufs=8, space="PSUM") as psp,
        ):
            # Resident rhs: both K-chunks of 2*realT shard, in per-i-tile
            # slices so the first matmul group starts early.
            rhs_sb = constp.tile([128, 2 * SHARD], bf16)
            nc.sync.dma_start(out=rhs_sb[:, 0:NT], in_=rhs[0:128, 0:NT])
            nc.sync.dma_start(out=rhs_sb[:, SHARD:SHARD + NT],
                              in_=rhs[128:256, 0:NT])
            # aug rhs rows: 0,1 = -hi/lo(|r_i|^2); 2,3 = 1.0; rest zero,
            # fully materialized host-side (zero-padded to K=128: a K<128
            # matmul stalls the PE pipeline; a device-side memset would
            # serialize the first aug matmul behind Pool)
            aug_sb = constp.tile([128, SHARD], bf16)
            nc.sync.dma_start(out=aug_sb[:, 0:NT], in_=aug[:, 0:NT])
            ones_sb = constp.tile([JTILE, JTILE], bf16)
            nc.sync.dma_start(out=ones_sb[:, :], in_=ones[:, :])

            realv = outp.tile([128, FJT * 8], f32)
            genv = outp.tile([128, NJT * 8], f32)
            geni = outp.tile([128, NJT * 8], u32)

            for jt in range(NJT):
                jo = jt * JTILE
                # spread the 60 real tiles evenly over the 96 slots so the
                # DVE load per slot is uniform
                do_real = (jt * RJT) // NJT != ((jt + 1) * RJT) // NJT
                rjt = (jt * RJT) // NJT
                jor = rjt * JTILE
                lhs_g = lhsp.tile([128, 2 * JTILE], bf16, tag="lhs_g")
                nc.sync.dma_start(
                    out=lhs_g[:, :].rearrange("p (c j) -> p c j", c=2),
                    in_=colg[:, jo:jo + JTILE].rearrange(
                        "(c p) j -> p c j", c=2),
                )
                if jt == 0:
                    # remaining const slices, behind jt0's critical loads
                    for it0 in range(1, NIT):
                        io0 = it0 * NT
                        nc.sync.dma_start(out=rhs_sb[:, io0:io0 + NT],
                                          in_=rhs[0:128, io0:io0 + NT])
                        nc.sync.dma_start(
                            out=rhs_sb[:, SHARD + io0:SHARD + io0 + NT],
                            in_=rhs[128:256, io0:io0 + NT])
                        nc.sync.dma_start(out=aug_sb[:, io0:io0 + NT],
                                          in_=aug[:, io0:io0 + NT])
                if do_real:
                    lhs_r = lhsp.tile([128, 2 * JTILE], bf16, tag="lhs_r")
                    nc.sync.dma_start(
                        out=lhs_r[:, :].rearrange("p (c j) -> p c j", c=2),
                        in_=colr[:, jor:jor + JTILE].rearrange(
                            "(c p) j -> p c j", c=2),
                    )
                    auglr_t = lhsp.tile([128, JTILE], bf16, tag="auglr_t")
                    nc.sync.dma_start(out=auglr_t[:, :],
                                      in_=auglr[:, jor:jor + JTILE])

                wide_g = widep.tile([128, SHARD], f32, tag="wide_g")
                if do_real:
                    wide_r = widep.tile([128, SHARD], f32, tag="wide_r")

                for it in range(NIT):
                    io = it * NT
                    jobs = [(lhs_g, ones_sb, wide_g)]
                    if do_real:
                        jobs.append((lhs_r, auglr_t, wide_r))
                    for lhs_t, aug_lhs, wide in jobs:
                        ps = psp.tile([128, NT], f32)
                        nc.tensor.matmul(
                            out=ps[:, :],
                            lhsT=lhs_t[:, 0:JTILE],
                            rhs=rhs_sb[:, io:io + NT],
                            start=True, stop=False,
                        )
                        nc.tensor.matmul(
                            out=ps[:, :],
                            lhsT=lhs_t[:, JTILE:2 * JTILE],
                            rhs=rhs_sb[:, SHARD + io:SHARD + io + NT],
                            start=False, stop=False,
                        )
                        nc.tensor.matmul(
                            out=ps[:, :],
                            lhsT=aug_lhs[:, :],
                            rhs=aug_sb[:, io:io + NT],
                            start=False, stop=True,
                        )
                        nc.scalar.activation(
                            out=wide[:, io:io + NT],
                            in_=ps[:, :],
                            func=mybir.ActivationFunctionType.Copy,
                        )

                nc.vector.max(out=genv[:, jt * 8:(jt + 1) * 8],
                              in_=wide_g[:, :])
                nc.vector.max_index(out=geni[:, jt * 8:(jt + 1) * 8],
                                    in_max=genv[:, jt * 8:(jt + 1) * 8],
                                    in_values=wide_g[:, :])
                if do_real:
                    if rjt < 12:
                        # self block: top-8 (host drops the diagonal top-1)
                        nc.vector.max(out=realv[:, rjt * 8:(rjt + 1) * 8],
                                      in_=wide_r[:, :])
                    elif rjt < FJT:
                        # only the max is needed: plain reduce is cheaper
                        nc.vector.tensor_reduce(
                            out=realv[:, rjt * 8:rjt * 8 + 1],
                            in_=wide_r[:, :],
                            axis=mybir.AxisListType.X,
                            op=mybir.AluOpType.max)
                    if PAR_LO <= rjt < PAR_HI:
                        par_t = parp.tile([128, SHARD], f32, tag="par_t")
                        nc.gpsimd.partition_all_reduce(
                            par_t[:, :], wide_r[:, :], channels=128,
                            reduce_op=bass_isa.ReduceOp.max)
                        nc.sync.dma_start(
                            out=o_par[rjt - PAR_LO:rjt - PAR_LO + 1, :],
                            in_=par_t[0:1, :])

            nc.sync.dma_start(out=o_realv[:, :], in_=realv[:, :])
            nc.sync.dma_start(out=o_genv[:, :], in_=genv[:, :])
            nc.sync.dma_start(out=o_geni[:, :], in_=geni[:, :])

    nc.compile()
    return nc


def _hilo(x, bf):
    hi = x.astype(bf)
    lo = (x - hi.astype(np.float32)).astype(bf)
    return hi, lo


def kernel(real_stats, gen_stats, _trace=False):
    import ml_dtypes
    from concourse.bass_utils import run_bass_kernel_spmd

    bf = ml_dtypes.bfloat16
    global _cached_nc
    real = np.ascontiguousarray(np.asarray(real_stats, dtype=np.float32))
    gen = np.ascontiguousarray(np.asarray(gen_stats, dtype=np.float32))

    realT = np.ascontiguousarray(real.T)                  # [D, N]
    genT = np.ascontiguousarray(gen.T)
    colg_bf = genT.astype(bf)
    rhs_bf = (2.0 * realT).astype(bf)                     # [D, N]
    b2 = np.sum(real.astype(np.float64) ** 2, axis=1).astype(np.float32)
    a2g = np.sum(gen.astype(np.float64) ** 2, axis=1).astype(np.float32)
    ones = np.zeros((JTILE, JTILE), dtype=bf)
    ones[0:2, :] = 1

    RW = RJT * JTILE                                      # 7680 rotated cols
    in_maps = []
    for c in range(NCORES):
        sl = slice(c * SHARD, (c + 1) * SHARD)
        negb2_hi, negb2_lo = _hilo(-b2[sl], bf)
        aug4 = np.zeros((128, SHARD), dtype=bf)
        aug4[0] = negb2_hi
        aug4[1] = negb2_lo
        aug4[2:4] = 1
        colr_rot = np.roll(realT, -c * SHARD, axis=1)[:, :RW]
        a2rot = np.roll(b2, -c * SHARD)[:RW]
        nega2_hi, nega2_lo = _hilo(-a2rot, bf)
        auglr = np.zeros((JTILE, RW), dtype=bf)
        auglr[0:2] = 1
        auglr[2] = nega2_hi
        auglr[3] = nega2_lo
        in_maps.append({
            "colr": colr_rot.astype(bf),
            "colg": colg_bf,
            "auglr": auglr,
            "rhs": np.ascontiguousarray(rhs_bf[:, sl]),
            "aug": aug4,
            "ones": ones,
        })

    if _cached_nc is None:
        _cached_nc = _build_nc()
    res = run_bass_kernel_spmd(_cached_nc, in_maps,
                               core_ids=list(range(NCORES)),
                               trace=_trace)

    # ---- host combine ----
    def grid(name, c, width):
        # [128, width*8] -> [128, width, 8]
        return res.results[c][name].reshape(128, width, 8)

    # real: Y = -dist^2 candidates, min-combined over all sources
    cand = np.full(N, np.inf, dtype=np.float64)
    p_idx = np.arange(128)
    for c in range(NCORES):
        rv = grid("o_realv", c, FJT)                      # [128, FJT, 8] of Y
        top1 = rv[:, :, 0]
        top2 = rv[:, :, 1]
        # self block (k < 12) contains the diagonal: Y_diag ~ 0, true
        # neighbors ~ -300 -> take top2 there when top1 is diag-like
        use2 = np.zeros((128, FJT), dtype=bool)
        use2[:, :12] = top1[:, :12] > -10.0
        y = np.where(use2, top2, top1)                    # [128, FJT]
        jglob = (c * SHARD + np.arange(FJT)[None, :] * JTILE
                 + p_idx[:, None]) % N
        np.minimum.at(cand, jglob.ravel(), (-y).ravel())
        par = res.results[c]["o_par"]                     # [48, SHARD] of Y
        par_min = -par.max(axis=0)                        # min dist^2 per i
        sl = slice(c * SHARD, (c + 1) * SHARD)
        cand[sl] = np.minimum(cand[sl], par_min)
    realNN = np.sqrt(np.maximum(cand, 0.0))               # [N]

    # gen: X = 2G - |r_i|^2;  d1^2 = a2g - max X
    j = np.arange(N)
    genv = np.stack([grid("o_genv", c, NJT)[:, :, 0] for c in range(NCORES)])
    geni = np.stack([res.results[c]["o_geni"].reshape(128, NJT, 8)[:, :, 0]
                     for c in range(NCORES)])
    # [8, 128, NJT] -> [8, N] with j = jt*128 + p
    gv = genv.transpose(0, 2, 1).reshape(NCORES, N)
    gi = geni.transpose(0, 2, 1).reshape(NCORES, N)
    cstar = gv.argmax(axis=0)
    d1 = np.sqrt(np.maximum(a2g - gv[cstar, j], 0.0))
    istar = cstar * SHARD + gi[cstar, j]
    d2 = realNN[istar]

    z = (d2 - d1) / 0.1
    authen = np.where(z >= 0, 1.0 / (1.0 + np.exp(-np.abs(z))),
                      np.exp(-np.abs(z)) / (1.0 + np.exp(-np.abs(z))))
    out = np.asarray(-100.0 * np.mean(authen), dtype=np.float32)
    if _trace:
        return out, res
    return out



# revision 4
# speedup vs baseline: 1.0588x; 1.0588x over previous
"""AuthPct metric kernel for 8 Trainium2 NeuronCores.

Distance-matrix layout (per core c): rows i = real shard c (1536), columns
j = 128-wide tiles; PE computes the Gram part 16*f_j.r_i into a 3-bank
[128,1536] PSUM tile (2 bf16 K=128 matmuls per 512-i bank, no aug pass on
the gen side).

The per-column argmin/min reductions run as ONE custom DVE op per tile
(PACK_BIAS / PACK_CLIP, registered at runtime into dve_ops.OPS):

    q = round(Src0)            # (x+1.5*2^23)-1.5*2^23 rounding trick
    P = q*2048 + Src1          # Src1 payload = round(-8|r_i|^2)*2048 + i
    accum_out = max_i(P)       # single 1x pass; PACK_CLIP also masks P>=thr

so one 1536-element scan yields both the quantized column max (d^2 to
1/16 resolution) and its argmax index in the low 11 bits.  The gen-side
-|r_i|^2 bias rides the payload tensor; real tiles get both norms from
the baseline-style aug matmul (hi/lo bf16 rows), the payload then only
carries the index.

real side (symmetric, shards c..c+4 rotated, like the baseline):
  m=0 self tiles:  PACK_CLIP (threshold kills the diagonal, Y_diag ~ 0
                   vs true neighbours ~ -8*300) -> j-side minima
  m=1..3:          j-side minima via Pool tensor_reduce on an ACT f32
                   PSUM->SBUF copy (complete Y values from the aug pass)
  m=1..4:          i-side minima via Pool partition_all_reduce on the
                   same copy; row 0 DMA'd out per tile
Free-side coverage t-3..t plus PAR coverage t+1..t+4 spans all 8 shards.

Host combine decodes q=floor(P/2048), idx=P mod 2048, takes exact fp32
PAR/Pool maxima as-is, min-combines across cores, gathers
d2 = realNN[argmin], sigmoid, mean.  Engine budget per core approx:
DVE 108 pack scans ~190us, Pool 36 reduce + 48 PAR ~190us, PE 972
matmuls ~210us, ACT 48 copies ~70us.
"""

import numpy as np

N = 12288
D = 256
NCORES = 8
SHARD = N // NCORES          # 1536 rows per core
JTILE = 128                  # j columns per tile (PSUM partitions)
NJT = N // JTILE             # 96 gen j-tiles
RJT = 60                     # real j-tiles: shards c..c+4 (rotated)
FJT = 48                     # real j-tiles with a j-side free scan (m=0..3)
NPAR = 48                    # real j-tiles with PAR harvest (m=1..4)
NT = 512                     # i elements per matmul (PSUM bank)
NIT = SHARD // NT            # 3 i-tiles

M_ROUND = 12582912.0         # 1.5*2^23
PSCALE = 2048.0
CLIP_THR = -131072.0         # -64*2048: packed threshold, d^2 > 8 required

J_SCAN_ON_POOL = False

_cached_nc = None
_pack_ops = None


def _register_pack_ops():
    """Register the PACK_BIAS/PACK_CLIP custom DVE ops (idempotent)."""
    global _pack_ops
    if _pack_ops is not None:
        return _pack_ops
    import concourse.dve_ops as dve_ops
    from concourse.dve_spec import (
        Spec, Src0, Src1, C0, C1, C2, MaxNeg, maxx, select, lower,
    )
    from concourse.dve_uop import DveOpSpec
    from concourse.dve_ops import has_src1

    if "PACK_BIAS_ANT" in dve_ops._SUB_OPCODE_FOR_NAME:
        by_name = {op.name: op for op in dve_ops.OPS}
        _pack_ops = (by_name["PACK_BIAS_ANT"], by_name["PACK_CLIP_ANT"])
        return _pack_ops

    FMIN = np.float32(-3.4028234663852886e38)

    def ref_bias(in0, in1, c0, c1, c2):
        x = np.asarray(in0, np.float32)
        c0 = np.float32(c0) if not isinstance(c0, np.ndarray) else c0.astype(np.float32)
        q = (np.float32(x + c0) - c0).astype(np.float32)
        P = (q * np.float32(c2) + np.asarray(in1, np.float32)).astype(np.float32)
        return P, P.max(axis=-1)

    def ref_clip(in0, in1, c0, c1, c2):
        x = np.asarray(in0, np.float32)
        c0 = np.float32(c0) if not isinstance(c0, np.ndarray) else c0.astype(np.float32)
        c1v = np.asarray(c1, np.float32)
        q = (np.float32(x + c0) - c0).astype(np.float32)
        P = (q * np.float32(c2) + np.asarray(in1, np.float32)).astype(np.float32)
        out = np.where(P < c1v, P, FMIN)
        return out, out.max(axis=-1)

    q = (Src0 + C0) - C0
    P = q * C2 + Src1
    spec_bias = Spec(body=P, accum=maxx, reference=ref_bias)
    spec_clip = Spec(body=select(P < C1, P, MaxNeg), accum=maxx,
                     reference=ref_clip)

    ops = []
    for name, spec in (("PACK_BIAS_ANT", spec_bias),
                       ("PACK_CLIP_ANT", spec_clip)):
        row = dve_ops._CUSTOM_DVE_ROW_BASE + len(dve_ops.OPS)
        dve_ops._SUB_OPCODE_FOR_NAME[name] = row
        shas = {}
        for ver in ("v3", "v4"):
            tmp = DveOpSpec(name=name, opcode=row, uops=lower(spec, ver=ver),
                            rd1_en=has_src1(spec))
            shas[ver] = tmp.sha(ver)
        op = dve_ops.DveOp(name, spec, subdim=False, uops_sha=shas)
        dve_ops.OPS.append(op)
        dve_ops.CUSTOM_DVE_SPECS[name] = spec
        ops.append(op)
    assert max(dve_ops._SUB_OPCODE_FOR_NAME.values()) < 0x20
    _pack_ops = tuple(ops)
    return _pack_ops


def _build_nc():
    import concourse.bass_isa as bass_isa
    import concourse.mybir as mybir
    from concourse import bacc
    from concourse.tile import TileContext

    PACK_BIAS, PACK_CLIP = _register_pack_ops()

    f32 = mybir.dt.float32
    bf16 = mybir.dt.bfloat16

    nc = bacc.Bacc("TRN2", target_bir_lowering=False, debug=False,
                   num_devices=NCORES)

    RW = RJT * JTILE
    colr = nc.dram_tensor("colr", [D, RW], bf16, kind="ExternalInput")
    colg = nc.dram_tensor("colg", [D, N], bf16, kind="ExternalInput")
    auglr = nc.dram_tensor("auglr", [JTILE, RW], bf16, kind="ExternalInput")
    rhs = nc.dram_tensor("rhs", [D, SHARD], bf16, kind="ExternalInput")
    augr = nc.dram_tensor("augr", [128, SHARD], bf16, kind="ExternalInput")
    payf = nc.dram_tensor("payf", [128, SHARD], f32, kind="ExternalInput")
    payi = nc.dram_tensor("payi", [128, SHARD], f32, kind="ExternalInput")

    o_gen = nc.dram_tensor("o_gen", [128, NJT], f32, kind="ExternalOutput")
    o_real = nc.dram_tensor("o_real", [128, FJT], f32, kind="ExternalOutput")
    o_par = nc.dram_tensor("o_par", [NPAR, SHARD], f32, kind="ExternalOutput")

    with TileContext(nc) as tc:
        with (
            tc.tile_pool(name="const", bufs=1) as constp,
            tc.tile_pool(name="lhs", bufs=6) as lhsp,
            tc.tile_pool(name="scr", bufs=3) as scrp,
            tc.tile_pool(name="pari", bufs=3) as parip,
            tc.tile_pool(name="paro", bufs=2) as parop,
            tc.tile_pool(name="outb", bufs=1) as outp,
            tc.tile_pool(name="ps", bufs=2, space="PSUM") as psp,
        ):
            # Resident rhs: both K-chunks of 16*realT shard, per-i-tile
            # slices so the first matmul group starts early.
            rhs_sb = constp.tile([128, 2 * SHARD], bf16)
            nc.sync.dma_start(out=rhs_sb[:, 0:NT], in_=rhs[0:128, 0:NT])
            nc.sync.dma_start(out=rhs_sb[:, SHARD:SHARD + NT],
                              in_=rhs[128:256, 0:NT])
            augr_sb = constp.tile([128, SHARD], bf16)
            nc.sync.dma_start(out=augr_sb[:, 0:NT], in_=augr[:, 0:NT])
            payf_sb = constp.tile([128, SHARD], f32)
            nc.sync.dma_start(out=payf_sb[:, :], in_=payf[:, :])
            payi_sb = constp.tile([128, SHARD], f32)
            nc.sync.dma_start(out=payi_sb[:, :], in_=payi[:, :])

            geno = outp.tile([128, NJT], f32)
            realo = outp.tile([128, FJT], f32)

            for jt in range(NJT):
                jo = jt * JTILE
                do_real = (jt * RJT) // NJT != ((jt + 1) * RJT) // NJT
                rjt = (jt * RJT) // NJT
                jor = rjt * JTILE
                lhs_g = lhsp.tile([128, 2 * JTILE], bf16, tag="lhs_g")
                nc.sync.dma_start(
                    out=lhs_g[:, :].rearrange("p (c j) -> p c j", c=2),
                    in_=colg[:, jo:jo + JTILE].rearrange(
                        "(c p) j -> p c j", c=2),
                )
                if jt == 0:
                    for it0 in range(1, NIT):
                        io0 = it0 * NT
                        nc.sync.dma_start(out=rhs_sb[:, io0:io0 + NT],
                                          in_=rhs[0:128, io0:io0 + NT])
                        nc.sync.dma_start(
                            out=rhs_sb[:, SHARD + io0:SHARD + io0 + NT],
                            in_=rhs[128:256, io0:io0 + NT])
                        nc.sync.dma_start(out=augr_sb[:, io0:io0 + NT],
                                          in_=augr[:, io0:io0 + NT])
                if do_real:
                    lhs_r = lhsp.tile([128, 2 * JTILE], bf16, tag="lhs_r")
                    nc.sync.dma_start(
                        out=lhs_r[:, :].rearrange("p (c j) -> p c j", c=2),
                        in_=colr[:, jor:jor + JTILE].rearrange(
                            "(c p) j -> p c j", c=2),
                    )
                    auglr_t = lhsp.tile([128, JTILE], bf16, tag="auglr_t")
                    nc.sync.dma_start(out=auglr_t[:, :],
                                      in_=auglr[:, jor:jor + JTILE])

                ps_g = psp.tile([128, SHARD], f32, tag="ps")
                for it in range(NIT):
                    io = it * NT
                    nc.tensor.matmul(
                        out=ps_g[:, io:io + NT],
                        lhsT=lhs_g[:, 0:JTILE],
                        rhs=rhs_sb[:, io:io + NT],
                        start=True, stop=False,
                    )
                    nc.tensor.matmul(
                        out=ps_g[:, io:io + NT],
                        lhsT=lhs_g[:, JTILE:2 * JTILE],
                        rhs=rhs_sb[:, SHARD + io:SHARD + io + NT],
                        start=False, stop=True,
                    )
                scr_g = scrp.tile([128, SHARD], f32, tag="scr")
                nc.vector._custom_dve(
                    PACK_BIAS, out=scr_g[:, :],
                    accum_out=geno[:, jt:jt + 1],
                    in0=ps_g[:, :], in1=payf_sb[:, :],
                    s0=M_ROUND, s1=0.0, imm2=PSCALE,
                )

                if do_real:
                    ps_r = psp.tile([128, SHARD], f32, tag="ps")
                    for it in range(NIT):
                        io = it * NT
                        nc.tensor.matmul(
                            out=ps_r[:, io:io + NT],
                            lhsT=lhs_r[:, 0:JTILE],
                            rhs=rhs_sb[:, io:io + NT],
                            start=True, stop=False,
                        )
                        nc.tensor.matmul(
                            out=ps_r[:, io:io + NT],
                            lhsT=lhs_r[:, JTILE:2 * JTILE],
                            rhs=rhs_sb[:, SHARD + io:SHARD + io + NT],
                            start=False, stop=False,
                        )
                        nc.tensor.matmul(
                            out=ps_r[:, io:io + NT],
                            lhsT=auglr_t[:, :],
                            rhs=augr_sb[:, io:io + NT],
                            start=False, stop=True,
                        )
                    if rjt < 12:
                        scr_r = scrp.tile([128, SHARD], f32, tag="scr")
                        nc.vector._custom_dve(
                            PACK_CLIP, out=scr_r[:, :],
                            accum_out=realo[:, rjt:rjt + 1],
                            in0=ps_r[:, :], in1=payi_sb[:, :],
                            s0=M_ROUND, s1=CLIP_THR, imm2=PSCALE,
                        )
                    else:
                        pin = parip.tile([128, SHARD], f32, tag="pin")
                        nc.scalar.activation(
                            out=pin[:, :], in_=ps_r[:, :],
                            func=mybir.ActivationFunctionType.Copy,
                        )
                        if rjt < FJT:
                            if J_SCAN_ON_POOL:
                                nc.gpsimd.tensor_reduce(
                                    out=realo[:, rjt:rjt + 1],
                                    in_=pin[:, :],
                                    axis=mybir.AxisListType.X,
                                    op=mybir.AluOpType.max)
                            else:
                                scr_r = scrp.tile([128, SHARD], f32,
                                                  tag="scr")
                                nc.vector._custom_dve(
                                    PACK_BIAS, out=scr_r[:, :],
                                    accum_out=realo[:, rjt:rjt + 1],
                                    in0=ps_r[:, :], in1=payi_sb[:, :],
                                    s0=M_ROUND, s1=0.0, imm2=PSCALE,
                                )
                        pout = parop.tile([128, SHARD], f32, tag="pout")
                        nc.gpsimd.partition_all_reduce(
                            pout[:, :], pin[:, :], channels=128,
                            reduce_op=bass_isa.ReduceOp.max)
                        nc.sync.dma_start(
                            out=o_par[rjt - 12:rjt - 11, :],
                            in_=pout[0:1, :])

            nc.sync.dma_start(out=o_gen[:, :], in_=geno[:, :])
            nc.sync.dma_start(out=o_real[:, :], in_=realo[:, :])

    nc.compile()
    return nc


def _hilo(x, dt):
    hi = x.astype(dt)
    lo = (x - hi.astype(np.float32)).astype(dt)
    return hi, lo


def kernel(real_stats, gen_stats, _trace=False):
    import ml_dtypes
    from concourse.bass_utils import run_bass_kernel_spmd

    bf = ml_dtypes.bfloat16
    global _cached_nc
    real = np.ascontiguousarray(np.asarray(real_stats, dtype=np.float32))
    gen = np.ascontiguousarray(np.asarray(gen_stats, dtype=np.float32))

    realT = np.ascontiguousarray(real.T)                  # [D, N]
    genT = np.ascontiguousarray(gen.T)
    colg_bf = genT.astype(bf)
    rhs_bf = (16.0 * realT).astype(bf)                    # [D, N]
    b2 = np.sum(real.astype(np.float64) ** 2, axis=1).astype(np.float32)
    a2g = np.sum(gen.astype(np.float64) ** 2, axis=1)

    RW = RJT * JTILE
    iota = np.arange(SHARD, dtype=np.float32)
    in_maps = []
    for c in range(NCORES):
        sl = slice(c * SHARD, (c + 1) * SHARD)
        negb2_hi, negb2_lo = _hilo(-8.0 * b2[sl], bf)
        augr_np = np.zeros((128, SHARD), dtype=bf)
        augr_np[0] = negb2_hi
        augr_np[1] = negb2_lo
        augr_np[2:4] = 1
        colr_rot = np.roll(realT, -c * SHARD, axis=1)[:, :RW]
        b2rot = np.roll(b2, -c * SHARD)[:RW]
        nega2_hi, nega2_lo = _hilo(-8.0 * b2rot, bf)
        auglr_np = np.zeros((JTILE, RW), dtype=bf)
        auglr_np[0:2] = 1
        auglr_np[2] = nega2_hi
        auglr_np[3] = nega2_lo
        payf_np = np.tile(
            (np.rint(-8.0 * b2[sl].astype(np.float64)).astype(np.float32)
             * np.float32(PSCALE) + iota), (128, 1))
        payi_np = np.tile(iota, (128, 1))
        in_maps.append({
            "colr": colr_rot.astype(bf),
            "colg": colg_bf,
            "auglr": auglr_np,
            "rhs": np.ascontiguousarray(rhs_bf[:, sl]),
            "augr": augr_np,
            "payf": np.ascontiguousarray(payf_np),
            "payi": np.ascontiguousarray(payi_np),
        })

    if _cached_nc is None:
        _cached_nc = _build_nc()
    res = run_bass_kernel_spmd(_cached_nc, in_maps,
                               core_ids=list(range(NCORES)),
                               trace=_trace)

    # ---- host combine (all f64) ----
    b2_64 = b2.astype(np.float64)
    cand = np.full(N, np.inf, dtype=np.float64)
    p_idx = np.arange(128)
    for c in range(NCORES):
        rv = res.results[c]["o_real"].astype(np.float64)  # [128, FJT]
        # self tiles (packed) and m=1..3 tiles
        if J_SCAN_ON_POOL:
            qv = np.empty_like(rv)
            qv[:, :12] = np.floor(rv[:, :12] / PSCALE)    # packed -> q
            qv[:, 12:] = rv[:, 12:] / PSCALE * PSCALE     # raw f32 max of Y8
            d2 = np.where(np.arange(FJT)[None, :] < 12,
                          -qv / 8.0, -rv / 8.0)
        else:
            qv = np.floor(rv / PSCALE)
            d2 = -qv / 8.0
        jglob = (c * SHARD + np.arange(FJT)[None, :] * JTILE
                 + p_idx[:, None]) % N
        np.minimum.at(cand, jglob.ravel(), d2.ravel())
        par = res.results[c]["o_par"].astype(np.float64)  # [NPAR, SHARD]
        par_d2 = -par.max(axis=0) / 8.0                   # min d^2 per i
        sl = slice(c * SHARD, (c + 1) * SHARD)
        cand[sl] = np.minimum(cand[sl], par_d2)
    realNN = np.sqrt(np.maximum(cand, 0.0))               # [N]

    # gen: packed P -> q (compare across cores), idx
    j = np.arange(N)
    P = np.stack([res.results[c]["o_gen"].astype(np.float64)
                  for c in range(NCORES)])                # [8, 128, NJT]
    # j = jt*128 + p  ->  [8, N]
    P = P.transpose(0, 2, 1).reshape(NCORES, N)
    q = np.floor(P / PSCALE)
    idx = (P - q * PSCALE).astype(np.int64)
    cstar = q.argmax(axis=0)
    d1 = np.sqrt(np.maximum(a2g - q[cstar, j] / 8.0, 0.0))
    istar = cstar * SHARD + idx[cstar, j]
    d2v = realNN[istar]

    z = (d2v - d1) / 0.1
    authen = np.where(z >= 0, 1.0 / (1.0 + np.exp(-np.abs(z))),
                      np.exp(-np.abs(z)) / (1.0 + np.exp(-np.abs(z))))
    out = np.asarray(-100.0 * np.mean(authen), dtype=np.float32)
    if _trace:
        return out, res
    return out


# revision 9
# speedup vs baseline: 1.0612x; 1.0023x over previous
"""AuthPct metric kernel for 8 Trainium2 NeuronCores.

Distance-matrix layout (per core c): rows i = real shard c (1536), columns
j = 128-wide tiles; PE computes the Gram part 16*f_j.r_i into a 3-bank
[128,1536] PSUM tile (2 bf16 K=128 matmuls per 512-i bank, no aug pass on
the gen side).

The per-column argmin/min reductions run as ONE custom DVE op per tile
(PACK_BIAS / PACK_CLIP, registered at runtime into dve_ops.OPS):

    q = round(Src0)            # (x+1.5*2^23)-1.5*2^23 rounding trick
    P = q*2048 + Src1          # Src1 payload = round(-8|r_i|^2)*2048 + i
    accum_out = max_i(P)       # single 1x pass; PACK_CLIP also masks P>=thr

so one 1536-element scan yields both the quantized column max (d^2 to
1/16 resolution) and its argmax index in the low 11 bits.  The gen-side
-|r_i|^2 bias rides the payload tensor; real tiles get both norms from
the baseline-style aug matmul (hi/lo bf16 rows), the payload then only
carries the index.

real side (symmetric, shards c..c+4 rotated, like the baseline):
  m=0 self tiles:  PACK_CLIP (threshold kills the diagonal, Y_diag ~ 0
                   vs true neighbours ~ -8*300) -> j-side minima
  m=1..3:          j-side minima via Pool tensor_reduce on an ACT f32
                   PSUM->SBUF copy (complete Y values from the aug pass)
  m=1..4:          i-side minima via Pool partition_all_reduce on the
                   same copy; row 0 DMA'd out per tile
Free-side coverage t-3..t plus PAR coverage t+1..t+4 spans all 8 shards.

Host combine decodes q=floor(P/2048), idx=P mod 2048, takes exact fp32
PAR/Pool maxima as-is, min-combines across cores, gathers
d2 = realNN[argmin], sigmoid, mean.  Engine budget per core approx:
DVE 108 pack scans ~190us, Pool 36 reduce + 48 PAR ~190us, PE 972
matmuls ~210us, ACT 48 copies ~70us.
"""

import numpy as np

N = 12288
D = 256
NCORES = 8
SHARD = N // NCORES          # 1536 rows per core
JTILE = 128                  # j columns per tile (PSUM partitions)
NJT = N // JTILE             # 96 gen j-tiles
RJT = 60                     # real j-tiles: shards c..c+4 (rotated)
FJT = 48                     # real j-tiles with a j-side free scan (m=0..3)
NPAR = 48                    # real j-tiles with PAR harvest (m=1..4)
NT = 512                     # i elements per matmul (PSUM bank)
NIT = SHARD // NT            # 3 i-tiles

M_ROUND = 12582912.0         # 1.5*2^23
PSCALE = 2048.0
CLIP_THR = -131072.0         # -64*2048: packed threshold, d^2 > 8 required

J_SCAN_ON_POOL = False

_cached_nc = None
_pack_ops = None


def _register_pack_ops():
    """Register the PACK_BIAS/PACK_CLIP custom DVE ops (idempotent)."""
    global _pack_ops
    if _pack_ops is not None:
        return _pack_ops
    import concourse.dve_ops as dve_ops
    from concourse.dve_spec import (
        Spec, Src0, Src1, C0, C1, C2, MaxNeg, maxx, select, lower,
    )
    from concourse.dve_uop import DveOpSpec
    from concourse.dve_ops import has_src1

    if "PACK_BIAS_ANT" in dve_ops._SUB_OPCODE_FOR_NAME:
        by_name = {op.name: op for op in dve_ops.OPS}
        _pack_ops = (by_name["PACK_BIAS_ANT"], by_name["PACK_CLIP_ANT"])
        return _pack_ops

    FMIN = np.float32(-3.4028234663852886e38)

    def ref_bias(in0, in1, c0, c1, c2):
        x = np.asarray(in0, np.float32)
        c0 = np.float32(c0) if not isinstance(c0, np.ndarray) else c0.astype(np.float32)
        q = (np.float32(x + c0) - c0).astype(np.float32)
        P = (q * np.float32(c2) + np.asarray(in1, np.float32)).astype(np.float32)
        return P, P.max(axis=-1)

    def ref_clip(in0, in1, c0, c1, c2):
        x = np.asarray(in0, np.float32)
        c0 = np.float32(c0) if not isinstance(c0, np.ndarray) else c0.astype(np.float32)
        c1v = np.asarray(c1, np.float32)
        q = (np.float32(x + c0) - c0).astype(np.float32)
        P = (q * np.float32(c2) + np.asarray(in1, np.float32)).astype(np.float32)
        out = np.where(P < c1v, P, FMIN)
        return out, out.max(axis=-1)

    q = (Src0 + C0) - C0
    P = q * C2 + Src1
    spec_bias = Spec(body=P, accum=maxx, reference=ref_bias)
    spec_clip = Spec(body=select(P < C1, P, MaxNeg), accum=maxx,
                     reference=ref_clip)

    ops = []
    for name, spec in (("PACK_BIAS_ANT", spec_bias),
                       ("PACK_CLIP_ANT", spec_clip)):
        row = dve_ops._CUSTOM_DVE_ROW_BASE + len(dve_ops.OPS)
        dve_ops._SUB_OPCODE_FOR_NAME[name] = row
        shas = {}
        for ver in ("v3", "v4"):
            tmp = DveOpSpec(name=name, opcode=row, uops=lower(spec, ver=ver),
                            rd1_en=has_src1(spec))
            shas[ver] = tmp.sha(ver)
        op = dve_ops.DveOp(name, spec, subdim=False, uops_sha=shas)
        dve_ops.OPS.append(op)
        dve_ops.CUSTOM_DVE_SPECS[name] = spec
        ops.append(op)
    assert max(dve_ops._SUB_OPCODE_FOR_NAME.values()) < 0x20
    _pack_ops = tuple(ops)
    return _pack_ops


def _build_nc():
    import concourse.bass_isa as bass_isa
    import concourse.mybir as mybir
    from concourse import bacc
    from concourse.tile import TileContext

    PACK_BIAS, PACK_CLIP = _register_pack_ops()

    f32 = mybir.dt.float32
    bf16 = mybir.dt.bfloat16

    nc = bacc.Bacc("TRN2", target_bir_lowering=False, debug=False,
                   num_devices=NCORES)

    RW = RJT * JTILE
    colr = nc.dram_tensor("colr", [D, RW], bf16, kind="ExternalInput")
    colg = nc.dram_tensor("colg", [D, N], bf16, kind="ExternalInput")
    auglr = nc.dram_tensor("auglr", [JTILE, RW], bf16, kind="ExternalInput")
    rhs = nc.dram_tensor("rhs", [D, SHARD], bf16, kind="ExternalInput")
    augr = nc.dram_tensor("augr", [128, SHARD], bf16, kind="ExternalInput")
    payf = nc.dram_tensor("payf", [128, SHARD], f32, kind="ExternalInput")
    payi = nc.dram_tensor("payi", [128, SHARD], f32, kind="ExternalInput")

    o_gen = nc.dram_tensor("o_gen", [128, NJT], f32, kind="ExternalOutput")
    o_real = nc.dram_tensor("o_real", [128, FJT], f32, kind="ExternalOutput")
    o_par = nc.dram_tensor("o_par", [NPAR, SHARD], f32, kind="ExternalOutput")

    with TileContext(nc) as tc:
        with (
            tc.tile_pool(name="const", bufs=1) as constp,
            tc.tile_pool(name="lhs", bufs=6) as lhsp,
            tc.tile_pool(name="scr", bufs=3) as scrp,
            tc.tile_pool(name="pari", bufs=6) as parip,
            tc.tile_pool(name="paro", bufs=3) as parop,
            tc.tile_pool(name="outb", bufs=1) as outp,
            tc.tile_pool(name="ps", bufs=2, space="PSUM") as psp,
        ):
            # Resident rhs: both K-chunks of 16*realT shard, per-i-tile
            # slices so the first matmul group starts early.
            rhs_sb = constp.tile([128, 2 * SHARD], bf16)
            nc.sync.dma_start(out=rhs_sb[:, 0:NT], in_=rhs[0:128, 0:NT])
            nc.sync.dma_start(out=rhs_sb[:, SHARD:SHARD + NT],
                              in_=rhs[128:256, 0:NT])
            augr_sb = constp.tile([128, SHARD], bf16)
            nc.sync.dma_start(out=augr_sb[:, 0:NT], in_=augr[:, 0:NT])
            payf_sb = constp.tile([128, SHARD], f32)
            nc.sync.dma_start(out=payf_sb[:, :], in_=payf[:, :])
            payi_sb = constp.tile([128, SHARD], f32)
            nc.sync.dma_start(out=payi_sb[:, :], in_=payi[:, :])

            geno = outp.tile([128, NJT], f32)
            realo = outp.tile([128, FJT], f32)

            for jt in range(NJT):
                jo = jt * JTILE
                do_real = (jt * RJT) // NJT != ((jt + 1) * RJT) // NJT
                rjt = (jt * RJT) // NJT
                jor = rjt * JTILE
                lhs_g = lhsp.tile([128, 2 * JTILE], bf16, tag="lhs_g")
                nc.sync.dma_start(
                    out=lhs_g[:, :].rearrange("p (c j) -> p c j", c=2),
                    in_=colg[:, jo:jo + JTILE].rearrange(
                        "(c p) j -> p c j", c=2),
                )
                if jt == 0:
                    for it0 in range(1, NIT):
                        io0 = it0 * NT
                        nc.sync.dma_start(out=rhs_sb[:, io0:io0 + NT],
                                          in_=rhs[0:128, io0:io0 + NT])
                        nc.sync.dma_start(
                            out=rhs_sb[:, SHARD + io0:SHARD + io0 + NT],
                            in_=rhs[128:256, io0:io0 + NT])
                        nc.sync.dma_start(out=augr_sb[:, io0:io0 + NT],
                                          in_=augr[:, io0:io0 + NT])
                if do_real:
                    lhs_r = lhsp.tile([128, 2 * JTILE], bf16, tag="lhs_r")
                    nc.sync.dma_start(
                        out=lhs_r[:, :].rearrange("p (c j) -> p c j", c=2),
                        in_=colr[:, jor:jor + JTILE].rearrange(
                            "(c p) j -> p c j", c=2),
                    )
                    auglr_t = lhsp.tile([128, JTILE], bf16, tag="auglr_t")
                    nc.sync.dma_start(out=auglr_t[:, :],
                                      in_=auglr[:, jor:jor + JTILE])

                ps_g = psp.tile([128, SHARD], f32, tag="ps")
                for it in range(NIT):
                    io = it * NT
                    nc.tensor.matmul(
                        out=ps_g[:, io:io + NT],
                        lhsT=lhs_g[:, 0:JTILE],
                        rhs=rhs_sb[:, io:io + NT],
                        start=True, stop=False,
                    )
                    nc.tensor.matmul(
                        out=ps_g[:, io:io + NT],
                        lhsT=lhs_g[:, JTILE:2 * JTILE],
                        rhs=rhs_sb[:, SHARD + io:SHARD + io + NT],
                        start=False, stop=True,
                    )
                scr_g = scrp.tile([128, SHARD], f32, tag="scr")
                nc.vector._custom_dve(
                    PACK_BIAS, out=scr_g[:, :],
                    accum_out=geno[:, jt:jt + 1],
                    in0=ps_g[:, :], in1=payf_sb[:, :],
                    s0=M_ROUND, s1=0.0, imm2=PSCALE,
                )

                if do_real:
                    ps_r = psp.tile([128, SHARD], f32, tag="ps")
                    for it in range(NIT):
                        io = it * NT
                        nc.tensor.matmul(
                            out=ps_r[:, io:io + NT],
                            lhsT=lhs_r[:, 0:JTILE],
                            rhs=rhs_sb[:, io:io + NT],
                            start=True, stop=False,
                        )
                        nc.tensor.matmul(
                            out=ps_r[:, io:io + NT],
                            lhsT=lhs_r[:, JTILE:2 * JTILE],
                            rhs=rhs_sb[:, SHARD + io:SHARD + io + NT],
                            start=False, stop=False,
                        )
                        nc.tensor.matmul(
                            out=ps_r[:, io:io + NT],
                            lhsT=auglr_t[:, :],
                            rhs=augr_sb[:, io:io + NT],
                            start=False, stop=True,
                        )
                    if rjt < 12:
                        scr_r = scrp.tile([128, SHARD], f32, tag="scr")
                        nc.vector._custom_dve(
                            PACK_CLIP, out=scr_r[:, :],
                            accum_out=realo[:, rjt:rjt + 1],
                            in0=ps_r[:, :], in1=payi_sb[:, :],
                            s0=M_ROUND, s1=CLIP_THR, imm2=PSCALE,
                        )
                    else:
                        pin = parip.tile([128, SHARD], f32, tag="pin")
                        nc.scalar.activation(
                            out=pin[:, :], in_=ps_r[:, :],
                            func=mybir.ActivationFunctionType.Copy,
                        )
                        if rjt < FJT:
                            if J_SCAN_ON_POOL:
                                nc.gpsimd.tensor_reduce(
                                    out=realo[:, rjt:rjt + 1],
                                    in_=pin[:, :],
                                    axis=mybir.AxisListType.X,
                                    op=mybir.AluOpType.max)
                            else:
                                scr_r = scrp.tile([128, SHARD], f32,
                                                  tag="scr")
                                nc.vector._custom_dve(
                                    PACK_BIAS, out=scr_r[:, :],
                                    accum_out=realo[:, rjt:rjt + 1],
                                    in0=ps_r[:, :], in1=payi_sb[:, :],
                                    s0=M_ROUND, s1=0.0, imm2=PSCALE,
                                )
                        pout = parop.tile([128, SHARD], f32, tag="pout")
                        nc.gpsimd.partition_all_reduce(
                            pout[:, :], pin[:, :], channels=128,
                            reduce_op=bass_isa.ReduceOp.max)
                        nc.sync.dma_start(
                            out=o_par[rjt - 12:rjt - 11, :],
                            in_=pout[0:1, :])

            nc.sync.dma_start(out=o_gen[:, :], in_=geno[:, :])
            nc.sync.dma_start(out=o_real[:, :], in_=realo[:, :])

    nc.compile()
    return nc


def _hilo(x, dt):
    hi = x.astype(dt)
    lo = (x - hi.astype(np.float32)).astype(dt)
    return hi, lo


def kernel(real_stats, gen_stats, _trace=False):
    import ml_dtypes
    from concourse.bass_utils import run_bass_kernel_spmd

    bf = ml_dtypes.bfloat16
    global _cached_nc
    real = np.ascontiguousarray(np.asarray(real_stats, dtype=np.float32))
    gen = np.ascontiguousarray(np.asarray(gen_stats, dtype=np.float32))

    realT = np.ascontiguousarray(real.T)                  # [D, N]
    genT = np.ascontiguousarray(gen.T)
    colg_bf = genT.astype(bf)
    rhs_bf = (16.0 * realT).astype(bf)                    # [D, N]
    b2 = np.sum(real.astype(np.float64) ** 2, axis=1).astype(np.float32)
    a2g = np.sum(gen.astype(np.float64) ** 2, axis=1)

    RW = RJT * JTILE
    iota = np.arange(SHARD, dtype=np.float32)
    in_maps = []
    for c in range(NCORES):
        sl = slice(c * SHARD, (c + 1) * SHARD)
        negb2_hi, negb2_lo = _hilo(-8.0 * b2[sl], bf)
        augr_np = np.zeros((128, SHARD), dtype=bf)
        augr_np[0] = negb2_hi
        augr_np[1] = negb2_lo
        augr_np[2:4] = 1
        colr_rot = np.roll(realT, -c * SHARD, axis=1)[:, :RW]
        b2rot = np.roll(b2, -c * SHARD)[:RW]
        nega2_hi, nega2_lo = _hilo(-8.0 * b2rot, bf)
        auglr_np = np.zeros((JTILE, RW), dtype=bf)
        auglr_np[0:2] = 1
        auglr_np[2] = nega2_hi
        auglr_np[3] = nega2_lo
        payf_np = np.tile(
            (np.rint(-8.0 * b2[sl].astype(np.float64)).astype(np.float32)
             * np.float32(PSCALE) + iota), (128, 1))
        payi_np = np.tile(iota, (128, 1))
        in_maps.append({
            "colr": colr_rot.astype(bf),
            "colg": colg_bf,
            "auglr": auglr_np,
            "rhs": np.ascontiguousarray(rhs_bf[:, sl]),
            "augr": augr_np,
            "payf": np.ascontiguousarray(payf_np),
            "payi": np.ascontiguousarray(payi_np),
        })

    if _cached_nc is None:
        _cached_nc = _build_nc()
    res = run_bass_kernel_spmd(_cached_nc, in_maps,
                               core_ids=list(range(NCORES)),
                               trace=_trace)

    # ---- host combine (all f64) ----
    b2_64 = b2.astype(np.float64)
    cand = np.full(N, np.inf, dtype=np.float64)
    p_idx = np.arange(128)
    for c in range(NCORES):
        rv = res.results[c]["o_real"].astype(np.float64)  # [128, FJT]
        # self tiles (packed) and m=1..3 tiles
        if J_SCAN_ON_POOL:
            qv = np.empty_like(rv)
            qv[:, :12] = np.floor(rv[:, :12] / PSCALE)    # packed -> q
            qv[:, 12:] = rv[:, 12:] / PSCALE * PSCALE     # raw f32 max of Y8
            d2 = np.where(np.arange(FJT)[None, :] < 12,
                          -qv / 8.0, -rv / 8.0)
        else:
            qv = np.floor(rv / PSCALE)
            d2 = -qv / 8.0
        jglob = (c * SHARD + np.arange(FJT)[None, :] * JTILE
                 + p_idx[:, None]) % N
        np.minimum.at(cand, jglob.ravel(), d2.ravel())
        par = res.results[c]["o_par"].astype(np.float64)  # [NPAR, SHARD]
        par_d2 = -par.max(axis=0) / 8.0                   # min d^2 per i
        sl = slice(c * SHARD, (c + 1) * SHARD)
        cand[sl] = np.minimum(cand[sl], par_d2)
    realNN = np.sqrt(np.maximum(cand, 0.0))               # [N]

    # gen: packed P -> q (compare across cores), idx
    j = np.arange(N)
    P = np.stack([res.results[c]["o_gen"].astype(np.float64)
                  for c in range(NCORES)])                # [8, 128, NJT]
    # j = jt*128 + p  ->  [8, N]
    P = P.transpose(0, 2, 1).reshape(NCORES, N)
    q = np.floor(P / PSCALE)
    idx = (P - q * PSCALE).astype(np.int64)
    cstar = q.argmax(axis=0)
    d1 = np.sqrt(np.maximum(a2g - q[cstar, j] / 8.0, 0.0))
    istar = cstar * SHARD + idx[cstar, j]
    d2v = realNN[istar]

    z = (d2v - d1) / 0.1
    authen = np.where(z >= 0, 1.0 / (1.0 + np.exp(-np.abs(z))),
                      np.exp(-np.abs(z)) / (1.0 + np.exp(-np.abs(z))))
    out = np.asarray(-100.0 * np.mean(authen), dtype=np.float32)
    if _trace:
        return out, res
    return out


# revision 12
# speedup vs baseline: 1.1752x; 1.1074x over previous
"""AuthPct metric kernel for 8 Trainium2 NeuronCores.

Per core c: rows i = real shard c (1536, moving operand), columns j in
128-wide tiles.  The Gram term 16*f_j.r_i is computed by ONE fp8-e4m3
DoubleRow matmul per 512-i PSUM bank (K=256 packed [128,2,*]); real
tiles add the hi/lo-bf16 norm aug matmul (-8|r_i|^2 - 8|r_j|^2) like the
original baseline, so real PSUM holds complete -8*d^2.

gen side (96 tiles): a single custom DVE op (PACK_BIAS, registered into
dve_ops.OPS at runtime) scans each 3-bank [128,1536] f32 PSUM tile:

    q = round(Src0); P = q*2048 + Src1; accum_out = max_i(P)
    (Src1 payload = round(-8|r_i|^2)*2048 + i)

one 1x pass yields the quantized column max (d^2 to 1/16) AND its argmin
index in the low 11 bits -- replacing the old max + max_index two-pass.

real side (symmetric, shards c..c+4 rotated): every real tile is
evacuated per-bank by ACT to an fp16 SBUF copy; the j-side minima for
ALL 60 tiles (coverage sources t-4..t) use fp16 tensor_mask_reduce at
DVE 2x (the m=0 self tile masks out its diagonal via the wrap-around
start=d+1,end=d window); the i-side minima use Pool partition_all_reduce
on only the m=1..3 copies (sources t+1..t+3) -- 36 PARs instead of the
baseline's 48, since free-side coverage grew to 5 shards.

Host combine: decode q=floor(P/2048), idx=P mod 2048 for gen; real/PAR
values are plain fp16-rounded maxima of -8*d^2; min-combine across
cores, d2 = realNN[argmin], sigmoid, mean.

Approx engine budget per core: DVE 96 packs + 60 tmr ~ 220us, Pool 36
PAR ~ 195us, ACT 180 bank copies ~ 110us, PE 468 DR + 180 aug matmuls.
"""

import os
import numpy as np

NO_DR = bool(int(os.environ.get("V5_NO_DR", "0")))
NO_TMR = bool(int(os.environ.get("V5_NO_TMR", "0")))
NO_PAR = bool(int(os.environ.get("V5_NO_PAR", "0")))

N = 12288
D = 256
NCORES = 8
SHARD = N // NCORES          # 1536 rows per core
JTILE = 128                  # j columns per tile (PSUM partitions)
NJT = N // JTILE             # 96 gen j-tiles
RJT = 60                     # real j-tiles: shards c..c+4 (rotated)
NPAR = 36                    # real j-tiles with PAR harvest (m=1..3)
NT = 512                     # i elements per matmul (PSUM bank)
NIT = SHARD // NT            # 3 i-tiles

M_ROUND = 12582912.0         # 1.5*2^23
PSCALE = 2048.0
FMIN = -3.4028234663852886e38

_cached_nc = None
_pack_ops = None


def _register_pack_ops():
    """Register the PACK_BIAS custom DVE op (idempotent)."""
    global _pack_ops
    if _pack_ops is not None:
        return _pack_ops
    import concourse.dve_ops as dve_ops
    from concourse.dve_spec import (
        Spec, Src0, Src1, C0, C1, C2, MaxNeg, maxx, select, lower,
    )
    from concourse.dve_uop import DveOpSpec
    from concourse.dve_ops import has_src1

    if "PACK_BIAS_ANT" in dve_ops._SUB_OPCODE_FOR_NAME:
        by_name = {op.name: op for op in dve_ops.OPS}
        _pack_ops = (by_name["PACK_BIAS_ANT"],)
        return _pack_ops

    def ref_bias(in0, in1, c0, c1, c2):
        x = np.asarray(in0, np.float32)
        c0 = np.float32(c0) if not isinstance(c0, np.ndarray) else c0.astype(np.float32)
        q = (np.float32(x + c0) - c0).astype(np.float32)
        P = (q * np.float32(c2) + np.asarray(in1, np.float32)).astype(np.float32)
        return P, P.max(axis=-1)

    q = (Src0 + C0) - C0
    P = q * C2 + Src1
    spec_bias = Spec(body=P, accum=maxx, reference=ref_bias)

    ops = []
    for name, spec in (("PACK_BIAS_ANT", spec_bias),):
        row = dve_ops._CUSTOM_DVE_ROW_BASE + len(dve_ops.OPS)
        dve_ops._SUB_OPCODE_FOR_NAME[name] = row
        shas = {}
        for ver in ("v3", "v4"):
            tmp = DveOpSpec(name=name, opcode=row, uops=lower(spec, ver=ver),
                            rd1_en=has_src1(spec))
            shas[ver] = tmp.sha(ver)
        op = dve_ops.DveOp(name, spec, subdim=False, uops_sha=shas)
        dve_ops.OPS.append(op)
        dve_ops.CUSTOM_DVE_SPECS[name] = spec
        ops.append(op)
    assert max(dve_ops._SUB_OPCODE_FOR_NAME.values()) < 0x20
    _pack_ops = tuple(ops)
    return _pack_ops


def _build_nc():
    import concourse.bass_isa as bass_isa
    import concourse.mybir as mybir
    from concourse import bacc
    from concourse.tile import TileContext

    (PACK_BIAS,) = _register_pack_ops()

    f32 = mybir.dt.float32
    fp16 = mybir.dt.float16
    bf16 = mybir.dt.bfloat16
    fp8 = mybir.dt.float8e4

    nc = bacc.Bacc("TRN2", target_bir_lowering=False, debug=False,
                   num_devices=NCORES)

    RW = RJT * JTILE
    # fp8 lhs/rhs packed for DoubleRow: [p, s, x] with k = s*128+p
    gdt = bf16 if NO_DR else fp8
    colr = nc.dram_tensor("colr", [128, 2, RW], gdt, kind="ExternalInput")
    colg = nc.dram_tensor("colg", [128, 2, N], gdt, kind="ExternalInput")
    rhs = nc.dram_tensor("rhs", [128, 2, SHARD], gdt, kind="ExternalInput")
    auglr = nc.dram_tensor("auglr", [JTILE, RW], bf16, kind="ExternalInput")
    augr = nc.dram_tensor("augr", [128, SHARD], bf16, kind="ExternalInput")
    payf = nc.dram_tensor("payf", [128, SHARD], f32, kind="ExternalInput")
    mstart = nc.dram_tensor("mstart", [128, 12], f32, kind="ExternalInput")
    mend = nc.dram_tensor("mend", [128, 12], f32, kind="ExternalInput")

    o_gen = nc.dram_tensor("o_gen", [128, NJT], f32, kind="ExternalOutput")
    o_real = nc.dram_tensor("o_real", [128, RJT], f32, kind="ExternalOutput")
    o_par = nc.dram_tensor("o_par", [NPAR, SHARD], f32, kind="ExternalOutput")

    with TileContext(nc) as tc:
        with (
            tc.tile_pool(name="const", bufs=1) as constp,
            tc.tile_pool(name="lhs", bufs=6) as lhsp,
            tc.tile_pool(name="scr", bufs=3) as scrp,
            tc.tile_pool(name="pari", bufs=6) as parip,
            tc.tile_pool(name="paro", bufs=3) as parop,
            tc.tile_pool(name="outb", bufs=1) as outp,
            tc.tile_pool(name="psg", bufs=2, space="PSUM") as psgp,
            tc.tile_pool(name="psr", bufs=2, space="PSUM") as psrp,
        ):
            # Resident moving operand (fp8 DR layout) + aug rows + payload.
            rhs_sb = constp.tile([128, 2 * SHARD], gdt)
            rhs_v = rhs_sb[:, :].rearrange("p (s n) -> p s n", s=2)
            nc.sync.dma_start(out=rhs_v[:, :, 0:NT], in_=rhs[:, :, 0:NT])
            augr_sb = constp.tile([128, SHARD], bf16)
            nc.sync.dma_start(out=augr_sb[:, 0:NT], in_=augr[:, 0:NT])
            payf_sb = constp.tile([128, SHARD], f32)
            nc.sync.dma_start(out=payf_sb[:, :], in_=payf[:, :])
            mst_sb = constp.tile([128, 12], f32)
            nc.sync.dma_start(out=mst_sb[:, :], in_=mstart[:, :])
            men_sb = constp.tile([128, 12], f32)
            nc.sync.dma_start(out=men_sb[:, :], in_=mend[:, :])
            fullend_sb = constp.tile([128, 1], f32)
            nc.vector.memset(fullend_sb[:, :], float(SHARD))

            geno = outp.tile([128, NJT], f32)
            realo = outp.tile([128, RJT], f32)

            for jt in range(NJT):
                jo = jt * JTILE
                do_real = (jt * RJT) // NJT != ((jt + 1) * RJT) // NJT
                rjt = (jt * RJT) // NJT
                jor = rjt * JTILE
                lhs_g = lhsp.tile([128, 2 * JTILE], gdt, tag="lhs_g")
                lhs_g_v = lhs_g[:, :].rearrange("p (s m) -> p s m", s=2)
                nc.sync.dma_start(out=lhs_g_v[:, :, :],
                                  in_=colg[:, :, jo:jo + JTILE])
                if jt == 0:
                    for it0 in range(1, NIT):
                        io0 = it0 * NT
                        nc.sync.dma_start(out=rhs_v[:, :, io0:io0 + NT],
                                          in_=rhs[:, :, io0:io0 + NT])
                        nc.sync.dma_start(out=augr_sb[:, io0:io0 + NT],
                                          in_=augr[:, io0:io0 + NT])
                if do_real:
                    lhs_r = lhsp.tile([128, 2 * JTILE], gdt, tag="lhs_r")
                    lhs_r_v = lhs_r[:, :].rearrange("p (s m) -> p s m", s=2)
                    nc.sync.dma_start(out=lhs_r_v[:, :, :],
                                      in_=colr[:, :, jor:jor + JTILE])
                    auglr_t = lhsp.tile([128, JTILE], bf16, tag="auglr_t")
                    nc.sync.dma_start(out=auglr_t[:, :],
                                      in_=auglr[:, jor:jor + JTILE])

                # --- gen tile: one DR matmul per bank into a 3-bank tile
                ps_g = psgp.tile([128, SHARD], f32, tag="psg")
                for it in range(NIT):
                    io = it * NT
                    if NO_DR:
                        nc.tensor.matmul(
                            out=ps_g[:, io:io + NT],
                            lhsT=lhs_g_v[:, 0, :],
                            rhs=rhs_v[:, 0, io:io + NT],
                            start=True, stop=False,
                        )
                        nc.tensor.matmul(
                            out=ps_g[:, io:io + NT],
                            lhsT=lhs_g_v[:, 1, :],
                            rhs=rhs_v[:, 1, io:io + NT],
                            start=False, stop=True,
                        )
                    else:
                        nc.tensor.matmul(
                            out=ps_g[:, io:io + NT],
                            lhsT=lhs_g_v[:, :, :],
                            rhs=rhs_v[:, :, io:io + NT],
                            start=True, stop=True,
                            perf_mode=mybir.MatmulPerfMode.DoubleRow,
                        )
                scr_g = scrp.tile([128, SHARD], f32, tag="scr")
                nc.vector._custom_dve(
                    PACK_BIAS, out=scr_g[:, :],
                    accum_out=geno[:, jt:jt + 1],
                    in0=ps_g[:, :], in1=payf_sb[:, :],
                    s0=M_ROUND, s1=0.0, imm2=PSCALE,
                )

                if do_real:
                    # --- real tile: DR gram + bf16 aug per bank, ACT
                    # evacuates each bank to an fp16 copy
                    pin = parip.tile([128, SHARD], fp16, tag="pin")
                    for it in range(NIT):
                        io = it * NT
                        ps_r = psrp.tile([128, NT], f32, tag="psr")
                        if NO_DR:
                            nc.tensor.matmul(
                                out=ps_r[:, :],
                                lhsT=lhs_r_v[:, 0, :],
                                rhs=rhs_v[:, 0, io:io + NT],
                                start=True, stop=False,
                            )
                            nc.tensor.matmul(
                                out=ps_r[:, :],
                                lhsT=lhs_r_v[:, 1, :],
                                rhs=rhs_v[:, 1, io:io + NT],
                                start=False, stop=False,
                            )
                        else:
                            nc.tensor.matmul(
                                out=ps_r[:, :],
                                lhsT=lhs_r_v[:, :, :],
                                rhs=rhs_v[:, :, io:io + NT],
                                start=True, stop=False,
                                perf_mode=mybir.MatmulPerfMode.DoubleRow,
                            )
                        nc.tensor.matmul(
                            out=ps_r[:, :],
                            lhsT=auglr_t[:, :],
                            rhs=augr_sb[:, io:io + NT],
                            start=False, stop=True,
                        )
                        nc.scalar.activation(
                            out=pin[:, io:io + NT], in_=ps_r[:, :],
                            func=mybir.ActivationFunctionType.Copy,
                        )
                    # j-side minima: self tile masks its diagonal via the
                    # custom TENSOR_MASK_REDUCE wrap window; others use a
                    # plain 1x tensor_reduce
                    if rjt < 12:
                        from concourse.dve_ops import TENSOR_MASK_REDUCE
                        scr_r = scrp.tile([128, SHARD], fp16, tag="scrh")
                        nc.vector._custom_dve(
                            TENSOR_MASK_REDUCE,
                            out=scr_r[:, :],
                            accum_out=realo[:, rjt:rjt + 1],
                            in0=pin[:, :],
                            in1=men_sb[:, rjt:rjt + 1],
                            s0=mst_sb[:, rjt:rjt + 1],
                            s1=FMIN, imm2=1.0,
                        )
                    else:
                        nc.vector.tensor_reduce(
                            out=realo[:, rjt:rjt + 1], in_=pin[:, :],
                            axis=mybir.AxisListType.X,
                            op=mybir.AluOpType.max)
                    if 12 <= rjt < 12 + NPAR and not NO_PAR:
                        pout = parop.tile([128, SHARD], f32, tag="pout")
                        nc.gpsimd.partition_all_reduce(
                            pout[:, :], pin[:, :], channels=128,
                            reduce_op=bass_isa.ReduceOp.max)
                        nc.sync.dma_start(
                            out=o_par[rjt - 12:rjt - 11, :],
                            in_=pout[0:1, :])

            nc.sync.dma_start(out=o_gen[:, :], in_=geno[:, :])
            nc.sync.dma_start(out=o_real[:, :], in_=realo[:, :])

    nc.compile()
    return nc


def _hilo(x, dt):
    hi = x.astype(dt)
    lo = (x - hi.astype(np.float32)).astype(dt)
    return hi, lo


def _pack_dr(a):
    """[256, X] -> DoubleRow layout [128, 2, X] with k = s*128 + p."""
    import ml_dtypes
    dt = ml_dtypes.bfloat16 if NO_DR else ml_dtypes.float8_e4m3fn
    return np.ascontiguousarray(
        a.reshape(2, 128, a.shape[1]).transpose(1, 0, 2)).astype(dt)


def kernel(real_stats, gen_stats, _trace=False):
    import ml_dtypes
    from concourse.bass_utils import run_bass_kernel_spmd

    bf = ml_dtypes.bfloat16
    global _cached_nc
    real = np.ascontiguousarray(np.asarray(real_stats, dtype=np.float32))
    gen = np.ascontiguousarray(np.asarray(gen_stats, dtype=np.float32))

    realT = np.ascontiguousarray(real.T)                  # [D, N]
    genT = np.ascontiguousarray(gen.T)
    colg_f8 = _pack_dr(genT)                              # [128, 2, N]
    b2 = np.sum(real.astype(np.float64) ** 2, axis=1).astype(np.float32)
    a2g = np.sum(gen.astype(np.float64) ** 2, axis=1)

    RW = RJT * JTILE
    iota = np.arange(SHARD, dtype=np.float32)
    p_ar = np.arange(128, dtype=np.float32)
    in_maps = []
    for c in range(NCORES):
        sl = slice(c * SHARD, (c + 1) * SHARD)
        negb2_hi, negb2_lo = _hilo(-8.0 * b2[sl], bf)
        augr_np = np.zeros((128, SHARD), dtype=bf)
        augr_np[0] = negb2_hi
        augr_np[1] = negb2_lo
        augr_np[2:4] = 1
        colr_rot = np.roll(realT, -c * SHARD, axis=1)[:, :RW]
        b2rot = np.roll(b2, -c * SHARD)[:RW]
        nega2_hi, nega2_lo = _hilo(-8.0 * b2rot, bf)
        auglr_np = np.zeros((JTILE, RW), dtype=bf)
        auglr_np[0:2] = 1
        auglr_np[2] = nega2_hi
        auglr_np[3] = nega2_lo
        payf_np = np.tile(
            (np.rint(-8.0 * b2[sl].astype(np.float64)).astype(np.float32)
             * np.float32(PSCALE) + iota), (128, 1))
        # self-tile diagonal masks: exclude free position jor+p via the
        # wrap-around window [d+1, d)
        dpos = p_ar[:, None] + (np.arange(12, dtype=np.float32)
                                * JTILE)[None, :]
        mstart_np = np.ascontiguousarray(dpos + 1.0)
        mend_np = np.ascontiguousarray(dpos)
        in_maps.append({
            "colr": _pack_dr(colr_rot),
            "colg": colg_f8,
            "auglr": auglr_np,
            "rhs": _pack_dr(16.0 * realT[:, sl]),
            "augr": augr_np,
            "payf": np.ascontiguousarray(payf_np),
            "mstart": mstart_np,
            "mend": mend_np,
        })

    if _cached_nc is None:
        _cached_nc = _build_nc()
    res = run_bass_kernel_spmd(_cached_nc, in_maps,
                               core_ids=list(range(NCORES)),
                               trace=_trace)

    # ---- host combine (f64) ----
    cand = np.full(N, np.inf, dtype=np.float64)
    p_idx = np.arange(128)
    for c in range(NCORES):
        rv = res.results[c]["o_real"].astype(np.float64)  # [128, RJT]
        d2 = -rv / 8.0
        jglob = (c * SHARD + np.arange(RJT)[None, :] * JTILE
                 + p_idx[:, None]) % N
        np.minimum.at(cand, jglob.ravel(), d2.ravel())
        par = res.results[c]["o_par"].astype(np.float64)  # [NPAR, SHARD]
        par_d2 = -par.max(axis=0) / 8.0
        sl = slice(c * SHARD, (c + 1) * SHARD)
        cand[sl] = np.minimum(cand[sl], par_d2)
    realNN = np.sqrt(np.maximum(cand, 0.0))               # [N]

    j = np.arange(N)
    P = np.stack([res.results[c]["o_gen"].astype(np.float64)
                  for c in range(NCORES)])                # [8, 128, NJT]
    P = P.transpose(0, 2, 1).reshape(NCORES, N)           # j = jt*128+p
    q = np.floor(P / PSCALE)
    idx = (P - q * PSCALE).astype(np.int64)
    cstar = q.argmax(axis=0)
    d1 = np.sqrt(np.maximum(a2g - q[cstar, j] / 8.0, 0.0))
    istar = cstar * SHARD + idx[cstar, j]
    d2v = realNN[istar]

    z = (d2v - d1) / 0.1
    authen = np.where(z >= 0, 1.0 / (1.0 + np.exp(-np.abs(z))),
                      np.exp(-np.abs(z)) / (1.0 + np.exp(-np.abs(z))))
    out = np.asarray(-100.0 * np.mean(authen), dtype=np.float32)
    if _trace:
        return out, res
    return out


# revision 16
# speedup vs baseline: 1.4096x; 1.1994x over previous
"""AuthPct metric kernel for 8 Trainium2 NeuronCores.

Per core c: rows i = real shard c (1536, moving operand), columns j in
128-wide tiles.  The Gram term 16*f_j.r_i is computed by ONE fp8-e4m3
DoubleRow matmul per 512-i PSUM bank (K=256 packed [128,2,*]); real
tiles add the hi/lo-bf16 norm aug matmul (-8|r_i|^2 - 8|r_j|^2) like the
original baseline, so real PSUM holds complete -8*d^2.

gen side (96 tiles): a single custom DVE op (PACK_BIAS, registered into
dve_ops.OPS at runtime) scans each 3-bank [128,1536] f32 PSUM tile:

    q = round(Src0); P = q*2048 + Src1; accum_out = max_i(P)
    (Src1 payload = round(-8|r_i|^2)*2048 + i)

one 1x pass yields the quantized column max (d^2 to 1/16) AND its argmin
index in the low 11 bits -- replacing the old max + max_index two-pass.

real side (symmetric, shards c..c+4 rotated): every real tile is
evacuated per-bank by ACT to an fp16 SBUF copy; the j-side minima for
ALL 60 tiles (coverage sources t-4..t) use fp16 tensor_mask_reduce at
DVE 2x (the m=0 self tile masks out its diagonal via the wrap-around
start=d+1,end=d window); the i-side minima use Pool partition_all_reduce
on only the m=1..3 copies (sources t+1..t+3) -- 36 PARs instead of the
baseline's 48, since free-side coverage grew to 5 shards.

Host combine: decode q=floor(P/2048), idx=P mod 2048 for gen; real/PAR
values are plain fp16-rounded maxima of -8*d^2; min-combine across
cores, d2 = realNN[argmin], sigmoid, mean.

Approx engine budget per core: DVE 96 packs + 60 tmr ~ 220us, Pool 36
PAR ~ 195us, ACT 180 bank copies ~ 110us, PE 468 DR + 180 aug matmuls.
"""

import os
import numpy as np

NO_DR = bool(int(os.environ.get("V5_NO_DR", "0")))
NO_TMR = bool(int(os.environ.get("V5_NO_TMR", "0")))
NO_PAR = bool(int(os.environ.get("V5_NO_PAR", "0")))

N = 12288
D = 256
NCORES = 8
SHARD = N // NCORES          # 1536 rows per core
JTILE = 128                  # j columns per tile (PSUM partitions)
NJT = N // JTILE             # 96 gen j-tiles
RJT = 60                     # real j-tiles: shards c..c+4 (rotated)
NPAR = 36                    # real j-tiles with PAR harvest (m=1..3)
NT = 512                     # i elements per matmul (PSUM bank)
NIT = SHARD // NT            # 3 i-tiles

M_ROUND = 12582912.0         # 1.5*2^23
PSCALE = 2048.0
FMIN = -3.4028234663852886e38

_cached_nc = None
_pack_ops = None


def _register_pack_ops():
    """Register the PACK_BIAS custom DVE op (idempotent)."""
    global _pack_ops
    if _pack_ops is not None:
        return _pack_ops
    import concourse.dve_ops as dve_ops
    from concourse.dve_spec import (
        Spec, Src0, Src1, C0, C1, C2, MaxNeg, maxx, select, lower,
    )
    from concourse.dve_uop import DveOpSpec
    from concourse.dve_ops import has_src1

    if "PACK_BIAS_ANT" in dve_ops._SUB_OPCODE_FOR_NAME:
        by_name = {op.name: op for op in dve_ops.OPS}
        _pack_ops = (by_name["PACK_BIAS_ANT"],)
        return _pack_ops

    def ref_bias(in0, in1, c0, c1, c2):
        x = np.asarray(in0, np.float32)
        c0 = np.float32(c0) if not isinstance(c0, np.ndarray) else c0.astype(np.float32)
        q = (np.float32(x + c0) - c0).astype(np.float32)
        P = (q * np.float32(c2) + np.asarray(in1, np.float32)).astype(np.float32)
        return P, P.max(axis=-1)

    q = (Src0 + C0) - C0
    P = q * C2 + Src1
    spec_bias = Spec(body=P, accum=maxx, reference=ref_bias)

    ops = []
    for name, spec in (("PACK_BIAS_ANT", spec_bias),):
        row = dve_ops._CUSTOM_DVE_ROW_BASE + len(dve_ops.OPS)
        dve_ops._SUB_OPCODE_FOR_NAME[name] = row
        shas = {}
        for ver in ("v3", "v4"):
            tmp = DveOpSpec(name=name, opcode=row, uops=lower(spec, ver=ver),
                            rd1_en=has_src1(spec))
            shas[ver] = tmp.sha(ver)
        op = dve_ops.DveOp(name, spec, subdim=False, uops_sha=shas)
        dve_ops.OPS.append(op)
        dve_ops.CUSTOM_DVE_SPECS[name] = spec
        ops.append(op)
    assert max(dve_ops._SUB_OPCODE_FOR_NAME.values()) < 0x20
    _pack_ops = tuple(ops)
    return _pack_ops


def _build_nc():
    import concourse.bass_isa as bass_isa
    import concourse.mybir as mybir
    from concourse import bacc
    from concourse.tile import TileContext

    (PACK_BIAS,) = _register_pack_ops()

    f32 = mybir.dt.float32
    fp16 = mybir.dt.float16
    bf16 = mybir.dt.bfloat16
    fp8 = mybir.dt.float8e4

    nc = bacc.Bacc("TRN2", target_bir_lowering=False, debug=False,
                   num_devices=NCORES)

    RW = RJT * JTILE
    # fp8 lhs/rhs packed for DoubleRow: [p, s, x] with k = s*128+p
    gdt = bf16 if NO_DR else fp8
    colr = nc.dram_tensor("colr", [128, 2, RW], gdt, kind="ExternalInput")
    colg = nc.dram_tensor("colg", [128, 2, N], gdt, kind="ExternalInput")
    rhs = nc.dram_tensor("rhs", [128, 2, SHARD], gdt, kind="ExternalInput")
    auglr = nc.dram_tensor("auglr", [JTILE, RW], bf16, kind="ExternalInput")
    augr = nc.dram_tensor("augr", [128, SHARD], bf16, kind="ExternalInput")
    payf = nc.dram_tensor("payf", [128, SHARD], f32, kind="ExternalInput")
    mstart = nc.dram_tensor("mstart", [128, 12], f32, kind="ExternalInput")
    mend = nc.dram_tensor("mend", [128, 12], f32, kind="ExternalInput")

    o_gen = nc.dram_tensor("o_gen", [128, NJT], f32, kind="ExternalOutput")
    o_real = nc.dram_tensor("o_real", [128, RJT], f32, kind="ExternalOutput")
    o_par = nc.dram_tensor("o_par", [NPAR, SHARD], f32, kind="ExternalOutput")

    with TileContext(nc) as tc:
        with (
            tc.tile_pool(name="const", bufs=1) as constp,
            tc.tile_pool(name="lhs", bufs=10) as lhsp,
            tc.tile_pool(name="scr", bufs=4) as scrp,
            tc.tile_pool(name="pari", bufs=8) as parip,
            tc.tile_pool(name="paro", bufs=4) as parop,
            tc.tile_pool(name="outb", bufs=1) as outp,
            tc.tile_pool(name="psg", bufs=2, space="PSUM") as psgp,
            tc.tile_pool(name="psr", bufs=2, space="PSUM") as psrp,
        ):
            # Resident moving operand (fp8 DR layout) + aug rows + payload.
            rhs_sb = constp.tile([128, 2 * SHARD], gdt)
            rhs_v = rhs_sb[:, :].rearrange("p (s n) -> p s n", s=2)
            nc.sync.dma_start(out=rhs_v[:, :, 0:NT], in_=rhs[:, :, 0:NT])
            augr_sb = constp.tile([128, SHARD], bf16)
            nc.sync.dma_start(out=augr_sb[:, 0:NT], in_=augr[:, 0:NT])
            payf_sb = constp.tile([128, SHARD], f32)
            nc.sync.dma_start(out=payf_sb[:, :], in_=payf[:, :])
            mst_sb = constp.tile([128, 12], f32)
            nc.sync.dma_start(out=mst_sb[:, :], in_=mstart[:, :])
            men_sb = constp.tile([128, 12], f32)
            nc.sync.dma_start(out=men_sb[:, :], in_=mend[:, :])
            fullend_sb = constp.tile([128, 1], f32)
            nc.vector.memset(fullend_sb[:, :], float(SHARD))

            geno = outp.tile([128, NJT], f32)
            realo = outp.tile([128, RJT], f32)

            for jt in range(NJT):
                jo = jt * JTILE
                do_real = (jt * RJT) // NJT != ((jt + 1) * RJT) // NJT
                rjt = (jt * RJT) // NJT
                jor = rjt * JTILE
                lhs_g = lhsp.tile([128, 2 * JTILE], gdt, tag="lhs_g")
                lhs_g_v = lhs_g[:, :].rearrange("p (s m) -> p s m", s=2)
                nc.sync.dma_start(out=lhs_g_v[:, :, :],
                                  in_=colg[:, :, jo:jo + JTILE])
                if jt == 0:
                    for it0 in range(1, NIT):
                        io0 = it0 * NT
                        nc.sync.dma_start(out=rhs_v[:, :, io0:io0 + NT],
                                          in_=rhs[:, :, io0:io0 + NT])
                        nc.sync.dma_start(out=augr_sb[:, io0:io0 + NT],
                                          in_=augr[:, io0:io0 + NT])
                if do_real:
                    lhs_r = lhsp.tile([128, 2 * JTILE], gdt, tag="lhs_r")
                    lhs_r_v = lhs_r[:, :].rearrange("p (s m) -> p s m", s=2)
                    nc.sync.dma_start(out=lhs_r_v[:, :, :],
                                      in_=colr[:, :, jor:jor + JTILE])
                    auglr_t = lhsp.tile([128, JTILE], bf16, tag="auglr_t")
                    nc.sync.dma_start(out=auglr_t[:, :],
                                      in_=auglr[:, jor:jor + JTILE])

                # --- gen tile: one DR matmul per bank into a 3-bank tile
                ps_g = psgp.tile([128, SHARD], f32, tag="psg")
                for it in range(NIT):
                    io = it * NT
                    if NO_DR:
                        nc.tensor.matmul(
                            out=ps_g[:, io:io + NT],
                            lhsT=lhs_g_v[:, 0, :],
                            rhs=rhs_v[:, 0, io:io + NT],
                            start=True, stop=False,
                        )
                        nc.tensor.matmul(
                            out=ps_g[:, io:io + NT],
                            lhsT=lhs_g_v[:, 1, :],
                            rhs=rhs_v[:, 1, io:io + NT],
                            start=False, stop=True,
                        )
                    else:
                        nc.tensor.matmul(
                            out=ps_g[:, io:io + NT],
                            lhsT=lhs_g_v[:, :, :],
                            rhs=rhs_v[:, :, io:io + NT],
                            start=True, stop=True,
                            perf_mode=mybir.MatmulPerfMode.DoubleRow,
                        )
                scr_g = scrp.tile([128, SHARD], f32, tag="scr")
                nc.vector._custom_dve(
                    PACK_BIAS, out=scr_g[:, :],
                    accum_out=geno[:, jt:jt + 1],
                    in0=ps_g[:, :], in1=payf_sb[:, :],
                    s0=M_ROUND, s1=0.0, imm2=PSCALE,
                )

                if do_real:
                    # --- real tile: DR gram + bf16 aug per bank, ACT
                    # evacuates each bank to an fp16 copy
                    pin = parip.tile([128, SHARD], fp16, tag="pin")
                    for it in range(NIT):
                        io = it * NT
                        ps_r = psrp.tile([128, NT], f32, tag="psr")
                        if NO_DR:
                            nc.tensor.matmul(
                                out=ps_r[:, :],
                                lhsT=lhs_r_v[:, 0, :],
                                rhs=rhs_v[:, 0, io:io + NT],
                                start=True, stop=False,
                            )
                            nc.tensor.matmul(
                                out=ps_r[:, :],
                                lhsT=lhs_r_v[:, 1, :],
                                rhs=rhs_v[:, 1, io:io + NT],
                                start=False, stop=False,
                            )
                        else:
                            nc.tensor.matmul(
                                out=ps_r[:, :],
                                lhsT=lhs_r_v[:, :, :],
                                rhs=rhs_v[:, :, io:io + NT],
                                start=True, stop=False,
                                perf_mode=mybir.MatmulPerfMode.DoubleRow,
                            )
                        nc.tensor.matmul(
                            out=ps_r[:, :],
                            lhsT=auglr_t[:, :],
                            rhs=augr_sb[:, io:io + NT],
                            start=False, stop=True,
                        )
                        nc.scalar.activation(
                            out=pin[:, io:io + NT], in_=ps_r[:, :],
                            func=mybir.ActivationFunctionType.Copy,
                        )
                    # j-side minima: self tile masks its diagonal via the
                    # custom TENSOR_MASK_REDUCE wrap window; others use a
                    # plain 1x tensor_reduce
                    if rjt < 12:
                        from concourse.dve_ops import TENSOR_MASK_REDUCE
                        scr_r = scrp.tile([128, SHARD], fp16, tag="scrh")
                        nc.vector._custom_dve(
                            TENSOR_MASK_REDUCE,
                            out=scr_r[:, :],
                            accum_out=realo[:, rjt:rjt + 1],
                            in0=pin[:, :],
                            in1=men_sb[:, rjt:rjt + 1],
                            s0=mst_sb[:, rjt:rjt + 1],
                            s1=FMIN, imm2=1.0,
                        )
                    else:
                        nc.vector.tensor_reduce(
                            out=realo[:, rjt:rjt + 1], in_=pin[:, :],
                            axis=mybir.AxisListType.X,
                            op=mybir.AluOpType.max)
                    if 12 <= rjt < 12 + NPAR and not NO_PAR:
                        pout = parop.tile([128, SHARD], f32, tag="pout")
                        nc.gpsimd.partition_all_reduce(
                            pout[:, :], pin[:, :], channels=128,
                            reduce_op=bass_isa.ReduceOp.max)
                        nc.sync.dma_start(
                            out=o_par[rjt - 12:rjt - 11, :],
                            in_=pout[0:1, :])

            nc.sync.dma_start(out=o_gen[:, :], in_=geno[:, :])
            nc.sync.dma_start(out=o_real[:, :], in_=realo[:, :])

    nc.compile()
    return nc


def _hilo(x, dt):
    hi = x.astype(dt)
    lo = (x - hi.astype(np.float32)).astype(dt)
    return hi, lo


def _pack_dr(a):
    """[256, X] -> DoubleRow layout [128, 2, X] with k = s*128 + p."""
    import ml_dtypes
    dt = ml_dtypes.bfloat16 if NO_DR else ml_dtypes.float8_e4m3fn
    return np.ascontiguousarray(
        a.reshape(2, 128, a.shape[1]).transpose(1, 0, 2)).astype(dt)


def kernel(real_stats, gen_stats, _trace=False):
    import ml_dtypes
    from concourse.bass_utils import run_bass_kernel_spmd

    bf = ml_dtypes.bfloat16
    global _cached_nc
    real = np.ascontiguousarray(np.asarray(real_stats, dtype=np.float32))
    gen = np.ascontiguousarray(np.asarray(gen_stats, dtype=np.float32))

    realT = np.ascontiguousarray(real.T)                  # [D, N]
    genT = np.ascontiguousarray(gen.T)
    colg_f8 = _pack_dr(genT)                              # [128, 2, N]
    b2 = np.sum(real.astype(np.float64) ** 2, axis=1).astype(np.float32)
    a2g = np.sum(gen.astype(np.float64) ** 2, axis=1)

    RW = RJT * JTILE
    iota = np.arange(SHARD, dtype=np.float32)
    p_ar = np.arange(128, dtype=np.float32)
    in_maps = []
    for c in range(NCORES):
        sl = slice(c * SHARD, (c + 1) * SHARD)
        negb2_hi, negb2_lo = _hilo(-8.0 * b2[sl], bf)
        augr_np = np.zeros((128, SHARD), dtype=bf)
        augr_np[0] = negb2_hi
        augr_np[1] = negb2_lo
        augr_np[2:4] = 1
        colr_rot = np.roll(realT, -c * SHARD, axis=1)[:, :RW]
        b2rot = np.roll(b2, -c * SHARD)[:RW]
        nega2_hi, nega2_lo = _hilo(-8.0 * b2rot, bf)
        auglr_np = np.zeros((JTILE, RW), dtype=bf)
        auglr_np[0:2] = 1
        auglr_np[2] = nega2_hi
        auglr_np[3] = nega2_lo
        payf_np = np.tile(
            (np.rint(-8.0 * b2[sl].astype(np.float64)).astype(np.float32)
             * np.float32(PSCALE) + iota), (128, 1))
        # self-tile diagonal masks: exclude free position jor+p via the
        # wrap-around window [d+1, d)
        dpos = p_ar[:, None] + (np.arange(12, dtype=np.float32)
                                * JTILE)[None, :]
        mstart_np = np.ascontiguousarray(dpos + 1.0)
        mend_np = np.ascontiguousarray(dpos)
        in_maps.append({
            "colr": _pack_dr(colr_rot),
            "colg": colg_f8,
            "auglr": auglr_np,
            "rhs": _pack_dr(16.0 * realT[:, sl]),
            "augr": augr_np,
            "payf": np.ascontiguousarray(payf_np),
            "mstart": mstart_np,
            "mend": mend_np,
        })

    if _cached_nc is None:
        _cached_nc = _build_nc()
    res = run_bass_kernel_spmd(_cached_nc, in_maps,
                               core_ids=list(range(NCORES)),
                               trace=_trace)

    # ---- host combine (f64) ----
    cand = np.full(N, np.inf, dtype=np.float64)
    p_idx = np.arange(128)
    for c in range(NCORES):
        rv = res.results[c]["o_real"].astype(np.float64)  # [128, RJT]
        d2 = -rv / 8.0
        jglob = (c * SHARD + np.arange(RJT)[None, :] * JTILE
                 + p_idx[:, None]) % N
        np.minimum.at(cand, jglob.ravel(), d2.ravel())
        par = res.results[c]["o_par"].astype(np.float64)  # [NPAR, SHARD]
        par_d2 = -par.max(axis=0) / 8.0
        sl = slice(c * SHARD, (c + 1) * SHARD)
        cand[sl] = np.minimum(cand[sl], par_d2)
    realNN = np.sqrt(np.maximum(cand, 0.0))               # [N]

    j = np.arange(N)
    P = np.stack([res.results[c]["o_gen"].astype(np.float64)
                  for c in range(NCORES)])                # [8, 128, NJT]
    P = P.transpose(0, 2, 1).reshape(NCORES, N)           # j = jt*128+p
    q = np.floor(P / PSCALE)
    idx = (P - q * PSCALE).astype(np.int64)
    cstar = q.argmax(axis=0)
    d1 = np.sqrt(np.maximum(a2g - q[cstar, j] / 8.0, 0.0))
    istar = cstar * SHARD + idx[cstar, j]
    d2v = realNN[istar]

    z = (d2v - d1) / 0.1
    authen = np.where(z >= 0, 1.0 / (1.0 + np.exp(-np.abs(z))),
                      np.exp(-np.abs(z)) / (1.0 + np.exp(-np.abs(z))))
    out = np.asarray(-100.0 * np.mean(authen), dtype=np.float32)
    if _trace:
        return out, res
    return out
